# revision 7
# baseline (speedup 1.0000x reference)
"""ARMA2d Trainium2 kernel: conv3x3 (256->256) + per-channel circular AR
solve, data-parallel over batch across 8 NeuronCores.

Math: reference does y = conv3x3(x, w); then per channel c a circular
2D AR solve y <- ifft2(fft2(y)/fft2(a_c)) where a_c is a separable 3x3
circular filter derived from alpha. Since a_c is separable, the solve is
out[b,c] = Gh[c] @ y[b,c] @ Gw[c].T with Gh/Gw 64x64 circulant-inverse
matrices (exact, precomputed on host from alpha in float64).

Device work per core (4 images): conv as 18 shifted matmuls (2 ci-tiles
x 9 taps) accumulated in PSUM; AR as PE transposes + two matmuls per
channel batched over the 4 images (N=256).
"""
import sys
import numpy as np

if "/opt/trn_rl_repo" not in sys.path:
    sys.path.insert(0, "/opt/trn_rl_repo")

B, C, H, W = 32, 256, 64, 64
NCORES = 8
BP = B // NCORES  # images per core

_CACHE = {}
LAST_EXEC_NS = None


def _build_nc(use_f32r=True):
    from contextlib import ExitStack
    import concourse.tile as tile
    from concourse import mybir, bacc

    f32 = mybir.dt.float32
    DT = mybir.dt.float32r if use_f32r else f32

    nc = bacc.Bacc("TRN2", target_bir_lowering=False, debug=False,
                   num_devices=NCORES)
    xp_p = nc.declare_dram_parameter("xp", [BP, 2, 128, 66, 66], DT, isOutput=False)
    wt_p = nc.declare_dram_parameter("wt", [128, 3, 3, 2, 2, 128], DT, isOutput=False)
    gwt_p = nc.declare_dram_parameter("gwt", [64, C, 64], DT, isOutput=False)
    ght_p = nc.declare_dram_parameter("ght", [64, C, 64], DT, isOutput=False)
    id_p = nc.declare_dram_parameter("ident", [128, 128], DT, isOutput=False)
    out_p = nc.declare_dram_parameter("out", [BP, C, H, W], DT, isOutput=True)

    with tile.TileContext(nc) as tc, ExitStack() as ctx:
        consts = ctx.enter_context(tc.tile_pool(name="consts", bufs=1))
        w_sb = consts.tile([128, 3, 3, 2, 2, 128], DT)
        nc.sync.dma_start(w_sb[:], wt_p[:])
        id_sb = consts.tile([128, 128], DT)
        nc.sync.dma_start(id_sb[:], id_p[:])

        dram = ctx.enter_context(tc.tile_pool(name="dram", bufs=1, space="DRAM"))
        y_hbm = dram.tile([2, 128, BP, H, W], DT)  # [cot, cl, b, h, w]

        xpool = ctx.enter_context(tc.tile_pool(name="xpool", bufs=2))

        # ---- Phase 1: conv ----
        ypool = ctx.enter_context(tc.tile_pool(name="ypool", bufs=3))
        with tc.tile_pool(name="cpsum", bufs=4, space="PSUM") as cpsum:
            for b in range(BP):
                xts = []
                for cit in range(2):
                    xt = xpool.tile([128, 66, 66], DT, tag=f"x{cit}", name=f"x{cit}_{b}")
                    nc.sync.dma_start(xt[:], xp_p[b, cit])
                    xts.append(xt)
                for cot in range(2):
                    for rb in range(8):
                        ps = cpsum.tile([128, 8, 64], f32, tag="cps", name=f"ps_{b}_{cot}_{rb}")
                        k = 0
                        for cit in range(2):
                            for ky in range(3):
                                for kx in range(3):
                                    lhsT = w_sb[:, ky, kx, cit, cot, :]
                                    rhs = xts[cit][:, rb * 8 + ky: rb * 8 + ky + 8,
                                                   kx: kx + 64]
                                    nc.tensor.matmul(ps[:], lhsT, rhs,
                                                     start=(k == 0), stop=(k == 17))
                                    k += 1
                        ysb = ypool.tile([128, 8, 64], DT, tag="ysb", name=f"ysb_{b}_{cot}_{rb}")
                        nc.vector.tensor_copy(ysb[:], ps[:])
                        nc.sync.dma_start(y_hbm[cot, :, b, rb * 8:(rb + 1) * 8, :], ysb[:])

        # ---- Phase 2: AR solve ----
        CH = 32  # channels per G chunk
        gpool = ctx.enter_context(tc.tile_pool(name="gpool", bufs=2))
        arp = ctx.enter_context(tc.tile_pool(name="arp", bufs=3))
        psA = ctx.enter_context(tc.tile_pool(name="psA", bufs=2, space="PSUM"))
        psB = ctx.enter_context(tc.tile_pool(name="psB", bufs=2, space="PSUM"))
        psC = ctx.enter_context(tc.tile_pool(name="psC", bufs=2, space="PSUM"))
        psD = ctx.enter_context(tc.tile_pool(name="psD", bufs=2, space="PSUM"))
        id64 = id_sb[0:64, 0:64]

        for chunk in range(C // CH):
            gw_sb = gpool.tile([64, CH, 64], DT, tag="gw", name=f"gw_{chunk}")
            nc.sync.dma_start(gw_sb[:], gwt_p[:, chunk * CH:(chunk + 1) * CH, :])
            gh_sb = gpool.tile([64, CH, 64], DT, tag="gh", name=f"gh_{chunk}")
            nc.sync.dma_start(gh_sb[:], ght_p[:, chunk * CH:(chunk + 1) * CH, :])

            for p in range(CH // 2):
                cg0 = chunk * CH + 2 * p
                # load Y tiles: [h, b, w] per parity
                yts = []
                for par in range(2):
                    c = cg0 + par
                    yt = arp.tile([64, BP, 64], DT, tag=f"Y{par}", name=f"Y{par}_{c}")
                    nc.sync.dma_start(
                        yt[:], y_hbm[c >> 7, c & 127].transpose([1, 0, 2]))
                    yts.append(yt)
                # T1: Y^T per (par, b) -> [w, par, b, h]
                t1p = psA.tile([64, 2, BP, 64], f32, tag="t1", name=f"t1_{cg0}")
                for par in range(2):
                    for bb in range(BP):
                        nc.tensor.transpose(t1p[:, par, bb, :].bitcast(DT),
                                            yts[par][:, bb, :], id64)
                yT = arp.tile([64, 2, BP, 64], DT, tag="yT", name=f"yT_{cg0}")
                nc.vector.tensor_copy(yT[:], t1p[:])
                # mm1: P = Gw[c] applied -> [j, par, b, h]
                p2p = psB.tile([64, 2, BP, 64], f32, tag="p2", name=f"p2_{cg0}")
                for par in range(2):
                    nc.tensor.matmul(p2p[:, par], gw_sb[:, 2 * p + par, :],
                                     yT[:, par], start=True, stop=True)
                p2 = arp.tile([64, 2, BP, 64], DT, tag="p2s", name=f"p2s_{cg0}")
                nc.vector.tensor_copy(p2[:], p2p[:])
                # T2: P^T per (par, b) -> [h, par, b, j]
                ptp = psC.tile([64, 2, BP, 64], f32, tag="pt", name=f"pt_{cg0}")
                for par in range(2):
                    for bb in range(BP):
                        nc.tensor.transpose(ptp[:, par, bb, :].bitcast(DT),
                                            p2[:, par, bb, :], id64)
                pt = arp.tile([64, 2, BP, 64], DT, tag="pts", name=f"pts_{cg0}")
                nc.vector.tensor_copy(pt[:], ptp[:])
                # mm2: V = Gh[c] applied -> [i, par, b, j]
                vp = psD.tile([64, 2, BP, 64], f32, tag="v", name=f"v_{cg0}")
                for par in range(2):
                    nc.tensor.matmul(vp[:, par], gh_sb[:, 2 * p + par, :],
                                     pt[:, par], start=True, stop=True)
                vs = arp.tile([64, 2, BP, 64], DT, tag="vs", name=f"vs_{cg0}")
                nc.vector.tensor_copy(vs[:], vp[:])
                for par in range(2):
                    c = cg0 + par
                    nc.sync.dma_start(out_p[:, c].transpose([1, 0, 2]), vs[:, par])

    nc.compile()
    return nc


def _host_prep(x, w, alpha):
    # circulant-inverse matrices from alpha (float64 for stability)
    s = np.sin(-np.pi / 4.0)
    c = np.cos(-np.pi / 4.0)
    aw = np.zeros((2, 3), dtype=np.float64)
    aw[0, 0] = np.float32(c)
    aw[1, 0] = np.float32(-s)
    aw[0, -1] = np.float32(s)
    aw[1, -1] = np.float32(c)
    at = np.tanh(alpha.astype(np.float64))  # (C,1,2,2)
    A_xy = np.einsum("ckab,bj->ckaj", at, aw)
    A_xy[:, :, :, 1] = 1.0
    A_x = A_xy[:, 0, 0, :]  # taps along H
    A_y = A_xy[:, 0, 1, :]  # taps along W

    def circ_inv(taps):
        a = np.zeros((taps.shape[0], H), dtype=np.float64)
        a[:, -1] = taps[:, 0]
        a[:, 0] = taps[:, 1]
        a[:, 1] = taps[:, 2]
        F = np.fft.fft(a, axis=-1)
        g = np.real(np.fft.ifft(1.0 / F, axis=-1))
        idx = (np.arange(H)[:, None] - np.arange(H)[None, :]) % H
        return g[:, idx]  # (C,64,64): out = G @ y

    Gh = circ_inv(A_x)
    Gw = circ_inv(A_y)
    ght = np.ascontiguousarray(Gh.transpose(2, 0, 1)).astype(np.float32)  # [h,c,i]
    gwt = np.ascontiguousarray(Gw.transpose(2, 0, 1)).astype(np.float32)  # [w,c,j]

    # weights: [ci_l, ky, kx, cit, cot, co_l]
    wt = w.reshape(2, 128, 2, 128, 3, 3).transpose(3, 4, 5, 2, 0, 1)
    wt = np.ascontiguousarray(wt).astype(np.float32)

    # x shards, padded
    xr = x.reshape(NCORES, BP, 2, 128, H, W)
    xpad = np.zeros((NCORES, BP, 2, 128, 66, 66), dtype=np.float32)
    xpad[..., 1:65, 1:65] = xr

    ident = np.eye(128, dtype=np.float32)
    return xpad, wt, gwt, ght, ident


def kernel(x, w, alpha, _trace=False):
    global LAST_EXEC_NS
    from concourse.bass_utils import run_bass_kernel_spmd

    x = np.ascontiguousarray(np.asarray(x), dtype=np.float32)
    w = np.ascontiguousarray(np.asarray(w), dtype=np.float32)
    alpha = np.asarray(alpha).astype(np.float64)

    key = "nc"
    if key not in _CACHE:
        _CACHE[key] = _build_nc()
    nc = _CACHE[key]

    xpad, wt, gwt, ght, ident = _host_prep(x, w, alpha)
    in_maps = [
        {"xp": xpad[i], "wt": wt, "gwt": gwt, "ght": ght, "ident": ident}
        for i in range(NCORES)
    ]
    res = run_bass_kernel_spmd(nc, in_maps, core_ids=list(range(NCORES)),
                               trace=_trace)
    LAST_EXEC_NS = res.exec_time_ns
    _CACHE["last_res"] = res
    outs = [np.asarray(r["out"]) for r in res.results]
    return np.concatenate(outs, axis=0)


# revision 14
# speedup vs baseline: 1.0325x; 1.0325x over previous
"""ARMA2d Trainium2 kernel: conv3x3 (256->256) + per-channel circular AR
solve, data-parallel over batch across 8 NeuronCores.

Math: reference does y = conv3x3(x, w); then per channel c a circular
2D AR solve y <- ifft2(fft2(y)/fft2(a_c)) where a_c is a separable 3x3
circular filter derived from alpha. Since a_c is separable, the solve is
out[b,c] = Gh[c] @ y[b,c] @ Gw[c].T with Gh/Gw 64x64 circulant-inverse
matrices (exact, precomputed on host from alpha in float64).

Device work per core (4 images): conv as 18 shifted matmuls (2 ci-tiles
x 9 taps) accumulated in PSUM; AR as PE transposes + two matmuls per
channel batched over the 4 images (N=256).
"""
import sys
import numpy as np

if "/opt/trn_rl_repo" not in sys.path:
    sys.path.insert(0, "/opt/trn_rl_repo")

B, C, H, W = 32, 256, 64, 64
NCORES = 8
BP = B // NCORES  # images per core

_CACHE = {}
LAST_EXEC_NS = None


def _build_nc(use_f32r=True):
    from contextlib import ExitStack
    import concourse.tile as tile
    from concourse import mybir, bacc

    f32 = mybir.dt.float32
    DT = mybir.dt.float32r if use_f32r else f32

    nc = bacc.Bacc("TRN2", target_bir_lowering=False, debug=False,
                   num_devices=NCORES)
    xp_p = nc.declare_dram_parameter("xp", [BP, 2, 128, 66, 66], DT, isOutput=False)
    wt_p = nc.declare_dram_parameter("wt", [128, 3, 3, 2, 2, 128], DT, isOutput=False)
    gwt_p = nc.declare_dram_parameter("gwt", [128, C // 2, 128], DT, isOutput=False)
    ght_p = nc.declare_dram_parameter("ght", [64, C, 64], DT, isOutput=False)
    id_p = nc.declare_dram_parameter("ident", [128, 128], DT, isOutput=False)
    out_p = nc.declare_dram_parameter("out", [BP, C, H, W], DT, isOutput=True)

    with tile.TileContext(nc) as tc, ExitStack() as ctx:
        consts = ctx.enter_context(tc.tile_pool(name="consts", bufs=1))
        w_sb = consts.tile([128, 3, 3, 2, 2, 128], DT)
        nc.sync.dma_start(w_sb[:], wt_p[:])
        id_sb = consts.tile([128, 128], DT)
        nc.sync.dma_start(id_sb[:], id_p[:])

        dram = ctx.enter_context(tc.tile_pool(name="dram", bufs=1, space="DRAM"))
        y_hbm = dram.tile([2, 128, BP, H, W], DT)  # [cot, cl, b, h, w]

        xpool = ctx.enter_context(tc.tile_pool(name="xpool", bufs=2))

        # ---- Phase 1: conv ----
        ypool = ctx.enter_context(tc.tile_pool(name="ypool", bufs=3))
        with tc.tile_pool(name="cpsum", bufs=4, space="PSUM") as cpsum:
            for b in range(BP):
                xts = []
                for cit in range(2):
                    xt = xpool.tile([128, 66, 66], DT, tag=f"x{cit}", name=f"x{cit}_{b}")
                    nc.sync.dma_start(xt[:], xp_p[b, cit])
                    xts.append(xt)
                for cot in range(2):
                    for rb in range(8):
                        ps = cpsum.tile([128, 8, 64], f32, tag="cps", name=f"ps_{b}_{cot}_{rb}")
                        k = 0
                        for cit in range(2):
                            for ky in range(3):
                                for kx in range(3):
                                    lhsT = w_sb[:, ky, kx, cit, cot, :]
                                    rhs = xts[cit][:, rb * 8 + ky: rb * 8 + ky + 8,
                                                   kx: kx + 64]
                                    nc.tensor.matmul(ps[:], lhsT, rhs,
                                                     start=(k == 0), stop=(k == 17))
                                    k += 1
                        ysb = ypool.tile([128, 8, 64], DT, tag="ysb", name=f"ysb_{b}_{cot}_{rb}")
                        nc.vector.tensor_copy(ysb[:], ps[:])
                        nc.sync.dma_start(y_hbm[cot, :, b, rb * 8:(rb + 1) * 8, :], ysb[:])

        # ---- Phase 2: AR solve ----
        # Channel pairs stacked on partitions; ALL PE operands/outputs at
        # base partition 0 (HW faults otherwise):
        #  T1: [h,(c,w)=128] -> [(c,w)=128, h]         (full-height out)
        #  mm1 W-solve: block-diag(GwT[cA],GwT[cB]) K=128 -> [(c,j), (b,h)]
        #  T2: [(c,j)=128, h] -> [h, (c,j)=128]
        #  mm2 H-solve: per-channel GhT[c] K=64, rhs strided (moving op)
        CH = 32   # channels per G chunk
        QPC = CH // 4  # quads per chunk
        gpool = ctx.enter_context(tc.tile_pool(name="gpool", bufs=2))
        arp = ctx.enter_context(tc.tile_pool(name="arp", bufs=3))
        psA = ctx.enter_context(tc.tile_pool(name="psA", bufs=2, space="PSUM"))
        psB = ctx.enter_context(tc.tile_pool(name="psB", bufs=2, space="PSUM"))
        psC = ctx.enter_context(tc.tile_pool(name="psC", bufs=1, space="PSUM"))
        psD = ctx.enter_context(tc.tile_pool(name="psD", bufs=1, space="PSUM"))
        id64 = id_sb[0:64, 0:64]

        for chunk in range(C // CH):
            gwbd = gpool.tile([128, CH // 2, 128], DT, tag="gw", name=f"gw_{chunk}")
            nc.sync.dma_start(gwbd[:], gwt_p[:, chunk * (CH // 2):(chunk + 1) * (CH // 2), :])
            gh_sb = gpool.tile([64, CH, 64], DT, tag="gh", name=f"gh_{chunk}")
            nc.sync.dma_start(gh_sb[:], ght_p[:, chunk * CH:(chunk + 1) * CH, :])

            for qd in range(QPC):
                qbase = chunk * CH + qd * 4   # global channel base of quad
                # load Y: [h, (q2, b, c2, w)] ; per (q,c) one DMA
                yq = arp.tile([64, 2, BP, 2, 64], DT, tag="yq", name=f"yq_{qbase}")
                for q in range(2):
                    for par in range(2):
                        c = qbase + 2 * q + par
                        nc.sync.dma_start(
                            yq[:, q, :, par, :],
                            y_hbm[c >> 7, c & 127].transpose([1, 0, 2]))
                # T1 per (q,b): [64,(c,w)=128] -> [(c,w)=128, h=64]
                t1q = psA.tile([128, 2, BP, 64], f32, tag="t1", name=f"t1_{qbase}")
                for q in range(2):
                    for bb in range(BP):
                        nc.tensor.transpose(t1q[:, q, bb, :].bitcast(DT),
                                            yq[:, q, bb, :, :], id64)
                t1s = arp.tile([128, 2, BP, 64], DT, tag="t1s", name=f"t1s_{qbase}")
                nc.vector.tensor_copy(t1s[:], t1q[:])
                # mm1 block-diag per pair: [(c,w),(c,j)] @ [(c,w),(b,h)]
                p2q = psB.tile([128, 2, BP, 64], f32, tag="p2", name=f"p2_{qbase}")
                for q in range(2):
                    pl = (qbase - chunk * CH) // 2 + q
                    nc.tensor.matmul(p2q[:, q], gwbd[:, pl, :],
                                     t1s[:, q, :, :], start=True, stop=True)
                p2s = arp.tile([128, 2, BP, 64], DT, tag="p2s", name=f"p2s_{qbase}")
                nc.scalar.copy(p2s[:], p2q[:])
                # T2 per (q,b): [(c,j)=128, h=64] -> [h=64, (c,j)=128]
                t2q = psC.tile([64, 2, BP, 128], f32, tag="t2", name=f"t2_{qbase}")
                for q in range(2):
                    for bb in range(BP):
                        nc.tensor.transpose(t2q[:, q, bb, :].bitcast(DT),
                                            p2s[:, q, bb, :], id_sb[:])
                t2s = arp.tile([64, 2, BP, 128], DT, tag="t2s", name=f"t2s_{qbase}")
                nc.vector.tensor_copy(t2s[:], t2q[:])
                # mm2 per channel: GhT[c] K=64; rhs [h,(b,j)] strided
                vq = psD.tile([64, 2, 2, BP, 64], f32, tag="v", name=f"v_{qbase}")
                for q in range(2):
                    for par in range(2):
                        cl = (qbase - chunk * CH) + 2 * q + par
                        nc.tensor.matmul(
                            vq[:, q, par], gh_sb[:, cl, :],
                            t2s[:, q, :, par * 64:(par + 1) * 64],
                            start=True, stop=True)
                vs = arp.tile([64, 2, 2, BP, 64], DT, tag="vs", name=f"vs_{qbase}")
                nc.scalar.copy(vs[:], vq[:])
                for q in range(2):
                    for par in range(2):
                        c = qbase + 2 * q + par
                        nc.sync.dma_start(out_p[:, c].transpose([1, 0, 2]),
                                          vs[:, q, par])

    nc.compile()
    return nc


def _host_prep(x, w, alpha):
    # circulant-inverse matrices from alpha (float64 for stability)
    s = np.sin(-np.pi / 4.0)
    c = np.cos(-np.pi / 4.0)
    aw = np.zeros((2, 3), dtype=np.float64)
    aw[0, 0] = np.float32(c)
    aw[1, 0] = np.float32(-s)
    aw[0, -1] = np.float32(s)
    aw[1, -1] = np.float32(c)
    at = np.tanh(alpha.astype(np.float64))  # (C,1,2,2)
    A_xy = np.einsum("ckab,bj->ckaj", at, aw)
    A_xy[:, :, :, 1] = 1.0
    A_x = A_xy[:, 0, 0, :]  # taps along H
    A_y = A_xy[:, 0, 1, :]  # taps along W

    def circ_inv(taps):
        a = np.zeros((taps.shape[0], H), dtype=np.float64)
        a[:, -1] = taps[:, 0]
        a[:, 0] = taps[:, 1]
        a[:, 1] = taps[:, 2]
        F = np.fft.fft(a, axis=-1)
        g = np.real(np.fft.ifft(1.0 / F, axis=-1))
        idx = (np.arange(H)[:, None] - np.arange(H)[None, :]) % H
        return g[:, idx]  # (C,64,64): out = G @ y

    Gh = circ_inv(A_x)
    Gw = circ_inv(A_y)
    # gwt: block-diag pairs [128, C//2, 128]:
    # rows 0-63 x cols 0-63 = GwT[2p] ([w,j]); rows/cols 64-127 = GwT[2p+1]
    gwT = Gw.transpose(0, 2, 1).astype(np.float32)   # [c][w, j]
    gwt = np.zeros((128, C // 2, 128), dtype=np.float32)
    for p in range(C // 2):
        gwt[0:64, p, 0:64] = gwT[2 * p]
        gwt[64:128, p, 64:128] = gwT[2 * p + 1]
    # ght compact: [h, c, i]
    ght = np.ascontiguousarray(Gh.transpose(2, 0, 1)).astype(np.float32)

    # weights: [ci_l, ky, kx, cit, cot, co_l]
    wt = w.reshape(2, 128, 2, 128, 3, 3).transpose(3, 4, 5, 2, 0, 1)
    wt = np.ascontiguousarray(wt).astype(np.float32)

    # x shards, padded
    xr = x.reshape(NCORES, BP, 2, 128, H, W)
    xpad = np.zeros((NCORES, BP, 2, 128, 66, 66), dtype=np.float32)
    xpad[..., 1:65, 1:65] = xr

    ident = np.eye(128, dtype=np.float32)
    return xpad, wt, gwt, ght, ident


def kernel(x, w, alpha, _trace=False):
    global LAST_EXEC_NS
    from concourse.bass_utils import run_bass_kernel_spmd

    x = np.ascontiguousarray(np.asarray(x), dtype=np.float32)
    w = np.ascontiguousarray(np.asarray(w), dtype=np.float32)
    alpha = np.asarray(alpha).astype(np.float64)

    key = "nc"
    if key not in _CACHE:
        _CACHE[key] = _build_nc()
    nc = _CACHE[key]

    xpad, wt, gwt, ght, ident = _host_prep(x, w, alpha)
    in_maps = [
        {"xp": xpad[i], "wt": wt, "gwt": gwt, "ght": ght, "ident": ident}
        for i in range(NCORES)
    ]
    res = run_bass_kernel_spmd(nc, in_maps, core_ids=list(range(NCORES)),
                               trace=_trace)
    LAST_EXEC_NS = res.exec_time_ns
    _CACHE["last_res"] = res
    outs = [np.asarray(r["out"]) for r in res.results]
    return np.concatenate(outs, axis=0)


# revision 17
# speedup vs baseline: 1.2999x; 1.2590x over previous
"""ARMA2d Trainium2 kernel: conv3x3 (256->256) + per-channel circular AR
solve, data-parallel over batch across 8 NeuronCores.

Math: reference does y = conv3x3(x, w); then per channel c a circular
2D AR solve y <- ifft2(fft2(y)/fft2(a_c)) where a_c is a separable 3x3
circular filter derived from alpha. Since a_c is separable, the solve is
out[b,c] = Gh[c] @ y[b,c] @ Gw[c].T with Gh/Gw 64x64 circulant-inverse
matrices (exact, precomputed on host from alpha in float64).

Device work per core (4 images): conv as 18 shifted matmuls (2 ci-tiles
x 9 taps) accumulated in PSUM; AR as PE transposes + two matmuls per
channel batched over the 4 images (N=256).
"""
import sys
import numpy as np

if "/opt/trn_rl_repo" not in sys.path:
    sys.path.insert(0, "/opt/trn_rl_repo")

B, C, H, W = 32, 256, 64, 64
NCORES = 8
BP = B // NCORES  # images per core

_CACHE = {}
LAST_EXEC_NS = None


def _build_nc(use_f32r=True):
    from contextlib import ExitStack
    import concourse.tile as tile
    from concourse import mybir, bacc

    f32 = mybir.dt.float32
    DT = mybir.dt.float32r if use_f32r else f32

    nc = bacc.Bacc("TRN2", target_bir_lowering=False, debug=False,
                   num_devices=NCORES)
    xp_p = nc.declare_dram_parameter("xp", [BP, 2, 128, 66, 66], DT, isOutput=False)
    wt_p = nc.declare_dram_parameter("wt", [128, 3, 3, 2, 2, 128], DT, isOutput=False)
    gwt_p = nc.declare_dram_parameter("gwt", [128, C // 2, 128], DT, isOutput=False)
    ght_p = nc.declare_dram_parameter("ght", [64, C, 64], DT, isOutput=False)
    id_p = nc.declare_dram_parameter("ident", [128, 128], DT, isOutput=False)
    out_p = nc.declare_dram_parameter("out", [BP, C, H, W], DT, isOutput=True)

    with tile.TileContext(nc) as tc, ExitStack() as ctx:
        consts = ctx.enter_context(tc.tile_pool(name="consts", bufs=1))
        w_sb = consts.tile([128, 3, 3, 2, 2, 128], DT)
        nc.sync.dma_start(w_sb[:], wt_p[:])
        id_sb = consts.tile([128, 128], DT)
        nc.sync.dma_start(id_sb[:], id_p[:])

        dram = ctx.enter_context(tc.tile_pool(name="dram", bufs=1, space="DRAM"))
        y_hbm = dram.tile([2, 128, BP, H, W], DT)  # [cot, cl, b, h, w]

        xpool = ctx.enter_context(tc.tile_pool(name="xpool", bufs=2))

        # ---- Phase 1: conv ----
        ypool = ctx.enter_context(tc.tile_pool(name="ypool", bufs=3))
        with tc.tile_pool(name="cpsum", bufs=4, space="PSUM") as cpsum:
            for b in range(BP):
                xts = []
                for cit in range(2):
                    xt = xpool.tile([128, 66, 66], DT, tag=f"x{cit}", name=f"x{cit}_{b}")
                    nc.sync.dma_start(xt[:], xp_p[b, cit])
                    xts.append(xt)
                for cot in range(2):
                    for rb in range(8):
                        ps = cpsum.tile([128, 8, 64], f32, tag="cps", name=f"ps_{b}_{cot}_{rb}")
                        k = 0
                        for cit in range(2):
                            for ky in range(3):
                                for kx in range(3):
                                    lhsT = w_sb[:, ky, kx, cit, cot, :]
                                    rhs = xts[cit][:, rb * 8 + ky: rb * 8 + ky + 8,
                                                   kx: kx + 64]
                                    nc.tensor.matmul(ps[:], lhsT, rhs,
                                                     start=(k == 0), stop=(k == 17))
                                    k += 1
                        ysb = ypool.tile([128, 8, 64], DT, tag="ysb", name=f"ysb_{b}_{cot}_{rb}")
                        nc.vector.tensor_copy(ysb[:], ps[:])
                        nc.sync.dma_start(y_hbm[cot, :, b, rb * 8:(rb + 1) * 8, :], ysb[:])

        # ---- Phase 2: AR solve ----
        # Channel pairs stacked on partitions; ALL PE operands/outputs at
        # base partition 0 (HW faults otherwise):
        #  T1: [h,(c,w)=128] -> [(c,w)=128, h]         (full-height out)
        #  mm1 W-solve: block-diag(GwT[cA],GwT[cB]) K=128 -> [(c,j), (b,h)]
        #  T2: [(c,j)=128, h] -> [h, (c,j)=128]
        #  mm2 H-solve: per-channel GhT[c] K=64, rhs strided (moving op)
        CH = 32   # channels per G chunk
        QPC = CH // 4  # quads per chunk
        gpool = ctx.enter_context(tc.tile_pool(name="gpool", bufs=2))
        arp = ctx.enter_context(tc.tile_pool(name="arp", bufs=3))
        psA = ctx.enter_context(tc.tile_pool(name="psA", bufs=2, space="PSUM"))
        psB = ctx.enter_context(tc.tile_pool(name="psB", bufs=2, space="PSUM"))
        psC = ctx.enter_context(tc.tile_pool(name="psC", bufs=1, space="PSUM"))
        psD = ctx.enter_context(tc.tile_pool(name="psD", bufs=1, space="PSUM"))
        id64 = id_sb[0:64, 0:64]

        for chunk in range(C // CH):
            gwbd = gpool.tile([128, CH // 2, 128], DT, tag="gw", name=f"gw_{chunk}")
            nc.sync.dma_start(gwbd[:], gwt_p[:, chunk * (CH // 2):(chunk + 1) * (CH // 2), :])
            gh_sb = gpool.tile([64, CH, 64], DT, tag="gh", name=f"gh_{chunk}")
            nc.sync.dma_start(gh_sb[:], ght_p[:, chunk * CH:(chunk + 1) * CH, :])

            for qd in range(QPC):
                qbase = chunk * CH + qd * 4   # global channel base of quad
                # load Y: [h, (q2, b, c2, w)] ; per (q,c) one DMA
                yq = arp.tile([64, 2, BP, 2, 64], DT, tag="yq", name=f"yq_{qbase}")
                for q in range(2):
                    for par in range(2):
                        c = qbase + 2 * q + par
                        nc.gpsimd.dma_start(
                            yq[:, q, :, par, :],
                            y_hbm[c >> 7, c & 127].transpose([1, 0, 2]))
                # T1 per (q,b): [64,(c,w)=128] -> [(c,w)=128, h=64]
                t1q = psA.tile([128, 2, BP, 64], f32, tag="t1", name=f"t1_{qbase}")
                for q in range(2):
                    for bb in range(BP):
                        nc.tensor.transpose(t1q[:, q, bb, :].bitcast(DT),
                                            yq[:, q, bb, :, :], id64)
                t1s = arp.tile([128, 2, BP, 64], DT, tag="t1s", name=f"t1s_{qbase}")
                nc.scalar.copy(t1s[:], t1q[:])
                # mm1 block-diag per pair: [(c,w),(c,j)] @ [(c,w),(b,h)]
                p2q = psB.tile([128, 2, BP, 64], f32, tag="p2", name=f"p2_{qbase}")
                for q in range(2):
                    pl = (qbase - chunk * CH) // 2 + q
                    nc.tensor.matmul(p2q[:, q], gwbd[:, pl, :],
                                     t1s[:, q, :, :], start=True, stop=True)
                p2s = arp.tile([128, 2, BP, 64], DT, tag="p2s", name=f"p2s_{qbase}")
                nc.scalar.copy(p2s[:], p2q[:])
                # T2 per (q,b): [(c,j)=128, h=64] -> [h=64, (c,j)=128]
                t2q = psC.tile([64, 2, BP, 128], f32, tag="t2", name=f"t2_{qbase}")
                for q in range(2):
                    for bb in range(BP):
                        nc.tensor.transpose(t2q[:, q, bb, :].bitcast(DT),
                                            p2s[:, q, bb, :], id_sb[:])
                t2s = arp.tile([64, 2, 2, BP, 64], DT, tag="t2s", name=f"t2s_{qbase}")
                nc.vector.tensor_copy(
                    t2s[:].transpose([0, 1, 3, 2, 4]),
                    t2q[:].rearrange("h q b (p j) -> h q b p j", p=2))
                # mm2 per channel: GhT[c] K=64; rhs [h,(b,j)] contiguous
                vq = psD.tile([64, 2, 2, BP, 64], f32, tag="v", name=f"v_{qbase}")
                for q in range(2):
                    for par in range(2):
                        cl = (qbase - chunk * CH) + 2 * q + par
                        nc.tensor.matmul(
                            vq[:, q, par], gh_sb[:, cl, :],
                            t2s[:, q, par], start=True, stop=True)
                vs = arp.tile([64, 2, 2, BP, 64], DT, tag="vs", name=f"vs_{qbase}")
                nc.vector.tensor_copy(vs[:], vq[:])
                for q in range(2):
                    for par in range(2):
                        c = qbase + 2 * q + par
                        nc.sync.dma_start(out_p[:, c].transpose([1, 0, 2]),
                                          vs[:, q, par])

    nc.compile()
    return nc


def _host_prep(x, w, alpha):
    # circulant-inverse matrices from alpha (float64 for stability)
    s = np.sin(-np.pi / 4.0)
    c = np.cos(-np.pi / 4.0)
    aw = np.zeros((2, 3), dtype=np.float64)
    aw[0, 0] = np.float32(c)
    aw[1, 0] = np.float32(-s)
    aw[0, -1] = np.float32(s)
    aw[1, -1] = np.float32(c)
    at = np.tanh(alpha.astype(np.float64))  # (C,1,2,2)
    A_xy = np.einsum("ckab,bj->ckaj", at, aw)
    A_xy[:, :, :, 1] = 1.0
    A_x = A_xy[:, 0, 0, :]  # taps along H
    A_y = A_xy[:, 0, 1, :]  # taps along W

    def circ_inv(taps):
        a = np.zeros((taps.shape[0], H), dtype=np.float64)
        a[:, -1] = taps[:, 0]
        a[:, 0] = taps[:, 1]
        a[:, 1] = taps[:, 2]
        F = np.fft.fft(a, axis=-1)
        g = np.real(np.fft.ifft(1.0 / F, axis=-1))
        idx = (np.arange(H)[:, None] - np.arange(H)[None, :]) % H
        return g[:, idx]  # (C,64,64): out = G @ y

    Gh = circ_inv(A_x)
    Gw = circ_inv(A_y)
    # gwt: block-diag pairs [128, C//2, 128]:
    # rows 0-63 x cols 0-63 = GwT[2p] ([w,j]); rows/cols 64-127 = GwT[2p+1]
    gwT = Gw.transpose(0, 2, 1).astype(np.float32)   # [c][w, j]
    gwt = np.zeros((128, C // 2, 128), dtype=np.float32)
    for p in range(C // 2):
        gwt[0:64, p, 0:64] = gwT[2 * p]
        gwt[64:128, p, 64:128] = gwT[2 * p + 1]
    # ght compact: [h, c, i]
    ght = np.ascontiguousarray(Gh.transpose(2, 0, 1)).astype(np.float32)

    # weights: [ci_l, ky, kx, cit, cot, co_l]
    wt = w.reshape(2, 128, 2, 128, 3, 3).transpose(3, 4, 5, 2, 0, 1)
    wt = np.ascontiguousarray(wt).astype(np.float32)

    # x shards, padded
    xr = x.reshape(NCORES, BP, 2, 128, H, W)
    xpad = np.zeros((NCORES, BP, 2, 128, 66, 66), dtype=np.float32)
    xpad[..., 1:65, 1:65] = xr

    ident = np.eye(128, dtype=np.float32)
    return xpad, wt, gwt, ght, ident


def kernel(x, w, alpha, _trace=False):
    global LAST_EXEC_NS
    from concourse.bass_utils import run_bass_kernel_spmd

    x = np.ascontiguousarray(np.asarray(x), dtype=np.float32)
    w = np.ascontiguousarray(np.asarray(w), dtype=np.float32)
    alpha = np.asarray(alpha).astype(np.float64)

    key = "nc"
    if key not in _CACHE:
        _CACHE[key] = _build_nc()
    nc = _CACHE[key]

    xpad, wt, gwt, ght, ident = _host_prep(x, w, alpha)
    in_maps = [
        {"xp": xpad[i], "wt": wt, "gwt": gwt, "ght": ght, "ident": ident}
        for i in range(NCORES)
    ]
    res = run_bass_kernel_spmd(nc, in_maps, core_ids=list(range(NCORES)),
                               trace=_trace)
    LAST_EXEC_NS = res.exec_time_ns
    _CACHE["last_res"] = res
    outs = [np.asarray(r["out"]) for r in res.results]
    return np.concatenate(outs, axis=0)
